# revision 1
# baseline (speedup 1.0000x reference)
"""MinLSTM Trainium2 Bass kernel.

Linear-space MinLSTM (gates normalized to f+i=1, g>=0, h0=0):
F=sig(x@Wf.T+bf); I=sig(x@Wi.T+bi); f=F/(F+I);
g=max(h_pre+bh+0.5, sig(h_pre+bh)); h_t = f*h_{t-1} + (1-f)*g computed as
a subtract-scan h = f*h_prev - w with w=(f-1)*g.

Matmuls run in bf16 (HW-measured ~4.6 rows/cycle vs fp16 2.9, fp32r 3.8;
fp8+DoubleRow no faster). Gate pipeline and output are fp16 for mantissa
headroom (rel err 2.4e-3 vs reference). Gates accumulate into pair-wide
[128, 1024] SBUF tiles; the DVE chain (S, R, f, g, w, scan) runs once per
pair, halving DVE instruction/semaphore count; output DMAs move 2KB lines.
h_pre+bh2 drains from PSUM via a cheap second ACT op so g runs SBUF-only.

Sharding: 8 cores = 4 batches x 2 halves of the D=1024 output channels.
"""

import numpy as np

B, L, D = 4, 4096, 1024
E = 512
P = 128
NCHUNK = 512
PAIR = 2 * NCHUNK
N_L = L // NCHUNK
N_PAIR = L // PAIR
N_K = D // P
N_E = E // P
N_CORES = 8

_prog_cache = {}


def build_program(reps=1):
    key = ("nc", reps)
    if key in _prog_cache:
        return _prog_cache[key]

    import concourse.bass as bass  # noqa: F401
    import concourse.tile as tile
    from concourse import bacc, mybir
    from concourse.mybir import AluOpType as alu

    f32 = mybir.dt.float32
    bf16 = mybir.dt.bfloat16
    f16 = mybir.dt.float16
    sig = mybir.ActivationFunctionType.Sigmoid
    ident = mybir.ActivationFunctionType.Identity

    nc = bacc.Bacc("TRN2", target_bir_lowering=False, debug=False)

    xt = nc.dram_tensor("xt", [D, L], bf16, kind="ExternalInput").ap()
    wts = [
        nc.dram_tensor(n, [D, E], bf16, kind="ExternalInput").ap()
        for n in ("wft", "wit", "wht")
    ]
    biases = {
        n: nc.dram_tensor(n, [E, 1], f32, kind="ExternalInput").ap()
        for n in ("bf", "bi", "bh", "bh2")
    }
    ht = nc.dram_tensor("ht", [E, L], f16, kind="ExternalOutput").ap()

    with tile.TileContext(nc) as tc:
        with (
            tc.tile_pool(name="wpool", bufs=1) as wpool,
            tc.tile_pool(name="bpool", bufs=1) as bpool,
            tc.tile_pool(name="xpool", bufs=2) as xpool,
            tc.tile_pool(name="gpool", bufs=3) as gpool,
            tc.tile_pool(name="hpool", bufs=2) as hpool,
            tc.tile_pool(name="pspool", bufs=8, space="PSUM") as pspool,
        ):
            KH = N_K // 2
            wtile = []
            for w in range(3):
                halves = []
                for wh in range(2):
                    t = wpool.tile(
                        [P, KH * E], bf16, tag=f"w{w}_{wh}", name=f"w{w}_{wh}"
                    )
                    tv = t.rearrange("p (kb e) -> p kb e", kb=KH)
                    src = wts[w].rearrange("(kb p) e -> p kb e", kb=N_K)
                    nc.sync.dma_start(
                        out=tv, in_=src[:, wh * KH:(wh + 1) * KH, :]
                    )
                    halves.append(t)
                wtile.append(halves)

            def lhsT(w, kb, eb):
                t = wtile[w][kb // KH]
                base = (kb % KH) * E + eb * P
                return t[:, base:base + P]

            btile = {}
            for nm in ("bf", "bi", "bh", "bh2"):
                t = bpool.tile([P, N_E], f32, tag=nm, name=f"b_{nm}")
                nc.gpsimd.dma_start(
                    out=t[:],
                    in_=biases[nm].rearrange("(eb p) one -> p (eb one)", eb=N_E),
                )
                btile[nm] = t

            xt3 = xt.rearrange("(kb p) l -> p kb l", kb=N_K)
            h_prev = [None] * N_E

            for rep in range(reps):
              for pc in range(N_PAIR):
                # pair-wide gate tiles, written per half by ACT
                gF, gI, gSH, gHP = {}, {}, {}, {}
                for eb in range(N_E):
                    gF[eb] = gpool.tile([P, PAIR], f32, tag="F", bufs=3,
                                        name=f"F{pc}_{eb}")
                    gI[eb] = gpool.tile([P, PAIR], f32, tag="I", bufs=3,
                                        name=f"I{pc}_{eb}")
                    gSH[eb] = gpool.tile([P, PAIR], f16, tag="SH", bufs=3,
                                         name=f"SH{pc}_{eb}")
                    gHP[eb] = gpool.tile([P, PAIR], f16, tag="HP", bufs=3,
                                         name=f"HP{pc}_{eb}")

                for half in range(2):
                    lc = 2 * pc + half
                    lsl = slice(lc * NCHUNK, (lc + 1) * NCHUNK)
                    hsl = slice(half * NCHUNK, (half + 1) * NCHUNK)
                    xtile = xpool.tile(
                        [P, N_K * NCHUNK], bf16, tag="x", name=f"x{rep}_{lc}"
                    )
                    xv = xtile.rearrange("p (kb l) -> p kb l", kb=N_K)
                    nc.scalar.dma_start(out=xv, in_=xt3[:, :, lsl])

                    for w in range(3):
                        for eb in range(N_E):
                            ps = pspool.tile(
                                [P, NCHUNK], f32,
                                tag="psh" if w == 2 else "ps",
                                bufs=4,
                                name=f"ps{rep}_{lc}_{eb}_{w}",
                            )
                            for kb in range(N_K):
                                nc.tensor.matmul(
                                    ps[:],
                                    lhsT=lhsT(w, kb, eb),
                                    rhs=xtile[:, kb * NCHUNK:(kb + 1) * NCHUNK],
                                    start=(kb == 0),
                                    stop=(kb == N_K - 1),
                                )
                            beb = slice(eb, eb + 1)
                            if w == 0:
                                nc.scalar.activation(
                                    gF[eb][:, hsl], ps[:], sig,
                                    bias=btile["bf"][:, beb],
                                )
                            elif w == 1:
                                nc.scalar.activation(
                                    gI[eb][:, hsl], ps[:], sig,
                                    bias=btile["bi"][:, beb],
                                )
                            else:
                                nc.scalar.activation(
                                    gSH[eb][:, hsl], ps[:], sig,
                                    bias=btile["bh"][:, beb],
                                )
                                nc.scalar.activation(
                                    gHP[eb][:, hsl], ps[:], ident,
                                    bias=btile["bh2"][:, beb],
                                )

                for eb in range(N_E):
                    esl = slice(eb * P, (eb + 1) * P)
                    psl = slice(pc * PAIR, (pc + 1) * PAIR)
                    F, I, SH, HP = gF[eb], gI[eb], gSH[eb], gHP[eb]

                    S = gpool.tile([P, PAIR], f32, tag="S", name=f"S{pc}_{eb}")
                    R = gpool.tile([P, PAIR], f32, tag="R", name=f"R{pc}_{eb}")
                    f = gpool.tile([P, PAIR], f16, tag="f", name=f"f{pc}_{eb}")
                    g = gpool.tile([P, PAIR], f16, tag="g", name=f"g{pc}_{eb}")
                    w_ = gpool.tile([P, PAIR], f16, tag="w", name=f"w{pc}_{eb}")

                    nc.vector.tensor_tensor(S[:], F[:], I[:], op=alu.add)
                    nc.vector.reciprocal_approx_fast(R[:], S[:])
                    nc.vector.tensor_tensor(f[:], F[:], R[:], op=alu.mult)
                    nc.vector.tensor_tensor(g[:], HP[:], SH[:], op=alu.max)
                    nc.vector.scalar_tensor_tensor(
                        w_[:], f[:], 1.0, g[:], op0=alu.subtract, op1=alu.mult,
                    )

                    h = hpool.tile([P, PAIR], f16, tag=f"h{eb}", name=f"h{pc}_{eb}")
                    initial = 0.0 if pc == 0 else h_prev[eb][:, PAIR - 1:PAIR]
                    nc.vector.tensor_tensor_scan(
                        h[:], f[:], w_[:], initial, op0=alu.mult, op1=alu.subtract
                    )
                    h_prev[eb] = h

                    nc.sync.dma_start(out=ht[esl, psl], in_=h[:])

    nc.compile()
    _prog_cache[key] = nc
    return nc


def _in_maps(x, W_f, b_f, W_i, b_i, W_h, b_h):
    import ml_dtypes
    bf16 = ml_dtypes.bfloat16
    x = np.ascontiguousarray(x, dtype=np.float32)
    xts = [np.ascontiguousarray(x[b].T.astype(bf16)) for b in range(B)]
    maps = []
    for c in range(N_CORES):
        b, half = divmod(c, 2)
        e0 = half * E
        m = {
            "xt": xts[b],
            "wft": np.ascontiguousarray(W_f[e0:e0 + E, :].T.astype(bf16)),
            "wit": np.ascontiguousarray(W_i[e0:e0 + E, :].T.astype(bf16)),
            "wht": np.ascontiguousarray(W_h[e0:e0 + E, :].T.astype(bf16)),
            "bf": np.ascontiguousarray(b_f[e0:e0 + E].reshape(E, 1), dtype=np.float32),
            "bi": np.ascontiguousarray(b_i[e0:e0 + E].reshape(E, 1), dtype=np.float32),
            "bh": np.ascontiguousarray(b_h[e0:e0 + E].reshape(E, 1), dtype=np.float32),
            "bh2": np.ascontiguousarray(
                (b_h[e0:e0 + E] + 0.5).reshape(E, 1), dtype=np.float32
            ),
        }
        maps.append(m)
    return maps


def kernel(x, W_f, b_f, W_i, b_i, W_h, b_h, _trace=False):
    from concourse.bass_utils import run_bass_kernel_spmd

    nc = build_program()
    in_maps = _in_maps(x, W_f, b_f, W_i, b_i, W_h, b_h)
    res = run_bass_kernel_spmd(nc, in_maps, list(range(N_CORES)), trace=_trace)
    _prog_cache["last_result"] = res

    out = np.empty((B, L, D), dtype=np.float32)
    for c in range(N_CORES):
        b, half = divmod(c, 2)
        e0 = half * E
        out[b, :, e0:e0 + E] = res.results[c]["ht"].T.astype(np.float32)
    return out

